# revision 42
# baseline (speedup 1.0000x reference)
"""HGNN model kernel for Trainium2, 8-core SPMD.

Math (reference):
  e   = par0*par1 * (diag[:,None] * ego) @ W + ego          (per user/item block)
  t   = adj.T @ e
  h   = adj @ t
  out = LayerNorm(h) * gamma + beta + ego

Key structure:
  * adj is {0, a} and LayerNorm is scale-invariant -> panels ship as fp8e4
    {0,1} EXACTLY (half the f16 bytes); LN eps rescaled host-side.
  * e precomputed on host (f16, 1.3 MB); no on-device phase 0. It loads
    first on the sync HWDGE ring (lowest first-byte latency) so the first
    matmul starts within a few microseconds.
  * Panels ride THREE DMA rings as contiguous k-segments sized by measured
    ring rate (gpsimd/SWDGE ~190 GB/s head segment, sync/scalar ~75-130
    tails), so arrival tracks the k-consumption order.
  * ONE AllGather of the full t table (1.3 MB) between the phases. It costs
    ~35-45 us of ncfw latency, but the entire phase-2 panel stream (13.1 MB)
    prefetches inside that window, phase 2 afterwards is pure PE work, and a
    single trigger point minimizes rank skew (two chunked AGs serialize on
    the cc stream and jitter badly - measured slower).
  * PE column-tiling: even/odd k-tiles accumulate into partition halves
    0-63/64-127 of the same PSUM bank via separate moving-operand streams
    (~2x matmul throughput); halves are stacked into SBUF, transposed per
    128-block, and merged by one free-dim add (also yields the transpose
    needed for the t-shard/LayerNorm layouts).

Sharding: core c owns node rows S*c..S*(c+1) (S = 1280).
"""

import numpy as np
import ml_dtypes

import concourse.bass as bass
import concourse.bacc as bacc
import concourse.tile as tile
from concourse import bass_utils, mybir
from concourse.masks import make_identity

F32 = mybir.dt.float32
F16 = mybir.dt.float16
F8 = mybir.dt.float8e4

N = 10240
D = 64
NU = 4096
NCORES = 8
S = N // NCORES          # 1280 rows per core
KT = N // 128            # 80 global 128-row k-tiles
LT = S // 128            # 10 local 128-row tiles
LN_EPS = 1e-5

# Contiguous k-segments per ring: (engine_idx, k0, count, batch);
# engine 0 = gpsimd, 1 = sync, 2 = scalar.
SEG1 = [(0, 0, 56, 8), (2, 56, 12, 12), (1, 68, 12, 12)]
SEG2 = [(0, 0, 32, 8), (1, 32, 48, 8)]

_CACHE = {}
LAST_RUN = None  # BassKernelResults of the most recent execution (for test.py)


def _build():
    if "nc" in _CACHE:
        return _CACHE["nc"]

    nc = bacc.Bacc(
        "TRN2",
        target_bir_lowering=False,
        debug=False,
        enable_asserts=True,
        num_devices=NCORES,
    )

    p1 = nc.dram_tensor("p1", [N, S], F8, kind="ExternalInput")
    p2 = nc.dram_tensor("p2", [N, S], F8, kind="ExternalInput")
    e_pre = nc.dram_tensor("e_pre", [128, KT * D], F16, kind="ExternalInput")
    res_pb = nc.dram_tensor("res_pb", [128, LT * D], F32, kind="ExternalInput")
    gamma_b = nc.dram_tensor("gamma_b", [128, D], F32, kind="ExternalInput")
    eps_in = nc.dram_tensor("eps_in", [128, 1], F32, kind="ExternalInput")
    out = nc.dram_tensor("out", [S, D], F32, kind="ExternalOutput")

    ACCS = ((0, 512), (512, 512), (1024, 256))

    with tile.TileContext(nc) as tc:
        with (
            tc.tile_pool(name="const", bufs=1) as const,
            tc.tile_pool(name="p1g", bufs=2) as p1g,
            tc.tile_pool(name="p1s", bufs=2) as p1s,
            tc.tile_pool(name="p1c", bufs=2) as p1c,
            tc.tile_pool(name="p2g", bufs=2) as p2g,
            tc.tile_pool(name="p2s", bufs=2) as p2s,
            tc.tile_pool(name="work", bufs=2) as work,
            tc.tile_pool(name="psumT", bufs=2, space="PSUM") as psumT,
            tc.tile_pool(name="psumacc", bufs=1, space="PSUM") as psumacc,
            tc.tile_pool(name="dram", bufs=1, space="DRAM") as dram,
        ):
            ENGS = [nc.gpsimd, nc.sync, nc.scalar]

            # ---- constants; e FIRST on the empty sync HWDGE ring ----
            e_sb = const.tile([128, KT * D], F16)
            with tc.high_priority():
                nc.sync.dma_start(e_sb[:], e_pre.ap())
            res_sb = const.tile([128, LT * D], F32)
            nc.gpsimd.dma_start(res_sb[:], res_pb.ap())
            gamma_sb = const.tile([128, D], F32)
            nc.gpsimd.dma_start(gamma_sb[:], gamma_b.ap())
            eps_sb = const.tile([128, 1], F32)
            nc.gpsimd.dma_start(eps_sb[:], eps_in.ap())
            ident_sb = const.tile([128, 128], F16)
            make_identity(nc, ident_sb[:])

            def stream_panels(dram_t, segs, pools, name):
                """Issue per-ring contiguous k-segment DMAs; return k->AP."""
                kv = dram_t.ap().rearrange("(k p) j -> k p j", p=128)
                tiles = {}
                for ei, k0, count, batch in segs:
                    for b0 in range(k0, k0 + count, batch):
                        cnt = min(batch, k0 + count - b0)
                        pan = pools[ei].tile(
                            [128, cnt * S], F8, name=f"{name}{ei}"
                        )
                        ENGS[ei].dma_start(
                            pan[:].rearrange("p (t j) -> p t j", j=S),
                            kv[b0 : b0 + cnt].rearrange("k p j -> p k j"),
                        )
                        for t_i in range(cnt):
                            tiles[b0 + t_i] = pan[:, t_i * S : (t_i + 1) * S]
                assert len(tiles) == KT
                return tiles

            def acc_matmuls(acc, lhsT, pan, m):
                half = slice(0, 64) if m % 2 == 0 else slice(64, 128)
                for i, (off, w) in enumerate(ACCS):
                    nc.tensor.matmul(
                        acc[i][half, :w],
                        lhsT,
                        pan[:, off : off + w],
                        start=(m == 0),
                        stop=(m >= KT - 2),
                        skip_group_check=(m % 2 == 1),
                    )

            def drain(acc, dst, nblk):
                """Stack even/odd PSUM halves, transpose each 128-block and
                merge halves with one free-dim add into dst [128, nblk*64]."""
                st = []
                for i, (off, w) in enumerate(ACCS):
                    s_ = work.tile([128, w], F16, name=f"st{i}")
                    nc.vector.tensor_copy(s_[0:64, :], acc[i][0:64, :w])
                    nc.vector.tensor_copy(s_[64:128, :], acc[i][64:128, :w])
                    st.append(s_)
                for r in range(nblk):
                    i = min(r // 4, 2)
                    loc = r * 128 - (0, 512, 1024)[i]
                    pt = psumT.tile([128, 128], F16, name="ptr")
                    nc.tensor.transpose(
                        pt[:], st[i][:, loc : loc + 128], ident_sb[:]
                    )
                    ptx = work.tile([128, 128], F16, name="ptx")
                    nc.vector.tensor_copy(ptx[:], pt[:])
                    nc.vector.tensor_add(
                        dst[:, r * D : (r + 1) * D],
                        ptx[:, 0:64],
                        ptx[:, 64:128],
                    )

            # ---- phase 1: t_shard.T = e.T @ p1, col-tiled even/odd ----
            acc_t = [
                psumacc.tile([128, 512], F32, name=f"acct{i}") for i in range(3)
            ]
            # sim-only: the odd half's first matmul has start=False; on HW
            # has_written=0 makes it an overwrite, but CoreSim poisons fresh
            # PSUM with NaN — zero it so the += assertion passes.
            for i, (off, w) in enumerate(ACCS):
                nc.vector.memset(acc_t[i][64:128, :w], 0.0)
            pan1 = stream_panels(p1, SEG1, {0: p1g, 1: p1s, 2: p1c}, "p1")
            for k in range(KT):
                acc_matmuls(acc_t, e_sb[:, k * D : (k + 1) * D], pan1[k], k)

            # ---- drain + single AllGather of the full t table ----
            with tc.high_priority():
                tsh_sb = work.tile([128, LT * D], F16, name="tsh", bufs=1)
                drain(acc_t, tsh_sb, LT)
                bounce_in = dram.tile([128, LT * D], F16)
                nc.scalar.dma_start(bounce_in[:], tsh_sb[:])
                bounce_out = dram.tile(
                    [128 * NCORES, LT * D], F16, addr_space="Shared"
                )
                nc.gpsimd.collective_compute(
                    "AllGather",
                    mybir.AluOpType.bypass,
                    replica_groups=[list(range(NCORES))],
                    ins=[bounce_in.opt()],
                    outs=[bounce_out.opt()],
                )
                t_sb = const.tile([128, NCORES * LT * D], F16)
                nc.scalar.dma_start(
                    t_sb[:].rearrange("p (r f) -> p r f", r=NCORES),
                    bounce_out[:].rearrange("(r p) f -> p r f", p=128),
                )

            # ---- phase 2: h_shard.T = t.T @ p2 (natural tile order) ----
            acc_h = [
                psumacc.tile([128, 512], F32, name=f"acch{i}") for i in range(3)
            ]
            for i, (off, w) in enumerate(ACCS):
                nc.vector.memset(acc_h[i][64:128, :w], 0.0)
            pan2 = stream_panels(p2, SEG2, {0: p2g, 1: p2s}, "p2")
            for m in range(KT):
                acc_matmuls(acc_h, t_sb[:, m * D : (m + 1) * D], pan2[m], m)

            # ---- drain h + batched LayerNorm + residual ----
            h_sb = work.tile([128, LT * D], F32, name="hsb", bufs=1)
            drain(acc_h, h_sb, LT)

            h3 = h_sb[:].rearrange("p (r d) -> p r d", d=D)
            nmu = work.tile([128, LT], F32, name="nmu", bufs=1)
            nc.vector.reduce_sum(
                nmu[:], h3, axis=mybir.AxisListType.X, negate=True
            )
            nc.vector.tensor_scalar_mul(nmu[:], nmu[:], 1.0 / D)  # -mu
            hc_sb = work.tile([128, LT * D], F32, name="hc", bufs=1)
            hc3 = hc_sb[:].rearrange("p (r d) -> p r d", d=D)
            nmu3 = nmu[:].rearrange("p (r d) -> p r d", d=1)
            a_b, b_b = bass.broadcast_tensor_aps(h3, nmu3)
            nc.vector.tensor_tensor(hc3, a_b, b_b, op=mybir.AluOpType.add)
            sq_sb = work.tile([128, LT * D], F32, name="sq", bufs=1)
            nc.vector.tensor_mul(sq_sb[:], hc_sb[:], hc_sb[:])
            ssq = work.tile([128, LT], F32, name="ssq", bufs=1)
            nc.vector.reduce_sum(
                ssq[:],
                sq_sb[:].rearrange("p (r d) -> p r d", d=D),
                axis=mybir.AxisListType.X,
            )
            std = work.tile([128, LT], F32, name="std", bufs=1)
            nc.scalar.activation(
                std[:],
                ssq[:],
                mybir.ActivationFunctionType.Sqrt,
                bias=eps_sb[:],
                scale=1.0 / D,
            )
            rstd = work.tile([128, LT], F32, name="rstd", bufs=1)
            nc.vector.reciprocal(rstd[:], std[:])
            o_sb = work.tile([128, LT * D], F32, name="osb", bufs=1)
            o3 = o_sb[:].rearrange("p (r d) -> p r d", d=D)
            rstd3 = rstd[:].rearrange("p (r d) -> p r d", d=1)
            a_b, b_b = bass.broadcast_tensor_aps(hc3, rstd3)
            nc.vector.tensor_tensor(o3, a_b, b_b, op=mybir.AluOpType.mult)
            g3 = gamma_sb[:].rearrange("p (r d) -> p r d", r=1)
            a_b, b_b = bass.broadcast_tensor_aps(o3, g3)
            nc.vector.tensor_tensor(o3, a_b, b_b, op=mybir.AluOpType.mult)
            nc.vector.tensor_add(o_sb[:], o_sb[:], res_sb[:])
            nc.gpsimd.dma_start(
                out.ap().rearrange("(r p) d -> p r d", p=128), o3
            )

    nc.compile()
    _CACHE["nc"] = nc
    return nc


def kernel(
    ego_embeddings,
    adj,
    W_u,
    diag_u,
    par_u,
    W_i,
    diag_i,
    par_i,
    ln_gamma,
    ln_beta,
    trace=False,
):
    global LAST_RUN
    ego = np.ascontiguousarray(ego_embeddings, dtype=np.float32)
    adj = np.ascontiguousarray(adj, dtype=np.float32)
    W_u = np.asarray(W_u, np.float32)
    W_i = np.asarray(W_i, np.float32)
    diag_u = np.asarray(diag_u, np.float32)
    diag_i = np.asarray(diag_i, np.float32)
    gamma = np.asarray(ln_gamma, np.float32)
    beta = np.asarray(ln_beta, np.float32)

    # host phase-0: e = c * (diag*ego) @ W + ego  (42 MFLOP, trivial)
    e = np.empty((N, D), np.float32)
    cu = float(par_u[0]) * float(par_u[1])
    ci = float(par_i[0]) * float(par_i[1])
    e[:NU] = cu * ((diag_u[:, None] * ego[:NU]) @ W_u) + ego[:NU]
    e[NU:] = ci * ((diag_i[:, None] * ego[NU:]) @ W_i) + ego[NU:]
    e16 = e.astype(np.float16)
    e_pre = np.ascontiguousarray(
        e16.reshape(KT, 128, D).transpose(1, 0, 2)
    ).reshape(128, KT * D)

    # LayerNorm(h) is invariant to a global scale on h = adj @ (adj.T @ e):
    # ship adj normalized by its max so the {0, a} graph is EXACTLY {0, 1}
    # in fp8e4, and rescale eps to keep LN bit-faithful.
    scale = float(adj.max())
    if scale <= 0.0:
        scale = 1.0
    inv = np.float32(1.0 / scale)
    eps_dev = np.float32(LN_EPS / (scale * scale * scale * scale))
    adj8 = (adj * inv).astype(ml_dtypes.float8_e4m3)

    gamma_b = np.ascontiguousarray(np.broadcast_to(gamma, (128, D)))
    eps_b = np.full((128, 1), eps_dev, np.float32)

    in_maps = []
    for c in range(NCORES):
        rows = slice(c * S, (c + 1) * S)
        res = ego[rows] + beta[None, :]
        res_pb = np.ascontiguousarray(
            res.reshape(LT, 128, D).transpose(1, 0, 2)
        ).reshape(128, LT * D)
        in_maps.append(
            {
                "p1": np.ascontiguousarray(adj8[:, rows]),
                "p2": np.ascontiguousarray(adj8[rows, :].T),
                "e_pre": e_pre,
                "res_pb": res_pb,
                "gamma_b": gamma_b,
                "eps_in": eps_b,
            }
        )

    nc = _build()
    res = bass_utils.run_bass_kernel_spmd(
        nc, in_maps, core_ids=list(range(NCORES)), trace=trace
    )
    LAST_RUN = res
    return np.concatenate([res.results[c]["out"] for c in range(NCORES)], axis=0)


# revision 43
# speedup vs baseline: 1.0945x; 1.0945x over previous
"""HGNN model kernel for Trainium2, 8-core SPMD.

Math (reference):
  e   = par0*par1 * (diag[:,None] * ego) @ W + ego          (per user/item block)
  t   = adj.T @ e
  h   = adj @ t
  out = LayerNorm(h) * gamma + beta + ego

Key structure:
  * adj is {0, a} and LayerNorm is scale-invariant -> panels ship as fp8e4
    {0,1} EXACTLY (half the f16 bytes); LN eps rescaled host-side.
  * e precomputed on host (f16, 1.3 MB); no on-device phase 0. It loads
    first on the sync HWDGE ring (lowest first-byte latency) so the first
    matmul starts within a few microseconds.
  * Panels ride THREE DMA rings (gpsimd/SWDGE ~200 GB/s + sync/scalar
    HWDGE ~110 each) in k-interleaved weighted rounds so panel arrival
    tracks the k-consumption order.
  * Phase-1 j-split 1024/256: AG#1 (1.05 MB) fires at ~1/3 of the run and
    overlaps phase-1B + phase-2A's 64-tile matmul stream; AG#2 (262 KB)
    fires right after phase 1 and overlaps phase-2A. t-table loads ride
    the scalar ring's tail so no panel stream blocks on a collective.
  * PE column-tiling: even/odd k-tiles accumulate into partition halves
    0-63/64-127 of the same PSUM bank via separate moving-operand streams
    (~2x matmul throughput); halves are stacked into SBUF, transposed per
    128-block, and merged by one free-dim add (which also produces the
    transpose needed for the t-shard / LayerNorm layouts).

Sharding: core c owns node rows S*c..S*(c+1) (S = 1280).
"""

import numpy as np
import ml_dtypes

import concourse.bass as bass
import concourse.bacc as bacc
import concourse.tile as tile
from concourse import bass_utils, mybir
from concourse.masks import make_identity

F32 = mybir.dt.float32
F16 = mybir.dt.float16
F8 = mybir.dt.float8e4

N = 10240
D = 64
NU = 4096
NCORES = 8
S = N // NCORES          # 1280 rows per core
KT = N // 128            # 80 global 128-row k-tiles
LT = S // 128            # 10 local 128-row tiles
LN_EPS = 1e-5

JA = 1024                # phase-1 j-split: half A columns (8 local tiles)
JB = S - JA              # 256 (2 local tiles)
LTA = JA // 128          # 8
LTB = JB // 128          # 2

# phase-2 consumes t tiles in AllGather arrival order.
TILES_A = [10 * c + q for c in range(NCORES) for q in range(LTA)]
TILES_B = [10 * c + q for c in range(NCORES) for q in range(LTA, LT)]

# k-interleaved DMA rounds: (engine_idx, k-tile count) per round slice.
# engine 0 = gpsimd (fast SWDGE), 1 = sync, 2 = scalar.
R1 = [(0, 8), (1, 4), (2, 4)]          # phase-1 rounds: 16 k-tiles
R2 = [(0, 10), (1, 6)]                 # phase-2 rounds: 16 k-tiles (no scalar)

_CACHE = {}
LAST_RUN = None  # BassKernelResults of the most recent execution (for test.py)


def _build():
    if "nc" in _CACHE:
        return _CACHE["nc"]

    nc = bacc.Bacc(
        "TRN2",
        target_bir_lowering=False,
        debug=False,
        enable_asserts=True,
        num_devices=NCORES,
    )

    p1a = nc.dram_tensor("p1a", [N, JA], F8, kind="ExternalInput")
    p1b = nc.dram_tensor("p1b", [N, JB], F8, kind="ExternalInput")
    p2 = nc.dram_tensor("p2", [N, S], F8, kind="ExternalInput")
    e_pre = nc.dram_tensor("e_pre", [128, KT * D], F16, kind="ExternalInput")
    res_pb = nc.dram_tensor("res_pb", [128, LT * D], F32, kind="ExternalInput")
    gamma_b = nc.dram_tensor("gamma_b", [128, D], F32, kind="ExternalInput")
    eps_in = nc.dram_tensor("eps_in", [128, 1], F32, kind="ExternalInput")
    out = nc.dram_tensor("out", [S, D], F32, kind="ExternalOutput")

    with tile.TileContext(nc) as tc:
        with (
            tc.tile_pool(name="const", bufs=1) as const,
            tc.tile_pool(name="pAg", bufs=2) as pAg,
            tc.tile_pool(name="pAs", bufs=2) as pAs,
            tc.tile_pool(name="pAc", bufs=2) as pAc,
            tc.tile_pool(name="pBg", bufs=2) as pBg,
            tc.tile_pool(name="pBs", bufs=2) as pBs,
            tc.tile_pool(name="pBc", bufs=2) as pBc,
            tc.tile_pool(name="p2g", bufs=2) as p2g,
            tc.tile_pool(name="p2s", bufs=2) as p2s,
            tc.tile_pool(name="work", bufs=2) as work,
            tc.tile_pool(name="psumT", bufs=2, space="PSUM") as psumT,
            tc.tile_pool(name="psumacc", bufs=1, space="PSUM") as psumacc,
            tc.tile_pool(name="dram", bufs=1, space="DRAM") as dram,
        ):
            ENGS = [nc.gpsimd, nc.sync, nc.scalar]

            # ---- constants; e FIRST on the empty sync HWDGE ring ----
            e_sb = const.tile([128, KT * D], F16)
            with tc.high_priority():
                nc.sync.dma_start(e_sb[:], e_pre.ap())
            res_sb = const.tile([128, LT * D], F32)
            nc.gpsimd.dma_start(res_sb[:], res_pb.ap())
            gamma_sb = const.tile([128, D], F32)
            nc.gpsimd.dma_start(gamma_sb[:], gamma_b.ap())
            eps_sb = const.tile([128, 1], F32)
            nc.gpsimd.dma_start(eps_sb[:], eps_in.ap())
            ident_sb = const.tile([128, 128], F16)
            make_identity(nc, ident_sb[:])

            def stream_panels(dram_t, width, rounds, pools, name):
                """Issue k-interleaved panel DMAs; return k -> panel AP."""
                kv = dram_t.ap().rearrange("(k p) j -> k p j", p=128)
                total = sum(n for _, n in rounds)
                assert KT % total == 0
                sched = []  # (k0, n, engine_idx)
                base = 0
                while base < KT:
                    off = 0
                    for ei, cnt in rounds:
                        sched.append((base + off, cnt, ei))
                        off += cnt
                    base += total
                tiles = {}
                for k0, cnt, ei in sched:
                    pan = pools[ei].tile(
                        [128, cnt * width], F8, name=f"{name}{ei}"
                    )
                    ENGS[ei].dma_start(
                        pan[:].rearrange("p (t j) -> p t j", j=width),
                        kv[k0 : k0 + cnt].rearrange("k p j -> p k j"),
                    )
                    for t_i in range(cnt):
                        tiles[k0 + t_i] = pan[
                            :, t_i * width : (t_i + 1) * width
                        ]
                return tiles

            # ---- phase 1A: t[:,0:1024].T = e.T @ p1a, col-tiled even/odd ----
            accA = [psumacc.tile([128, 512], F32, name=f"accA{i}") for i in range(2)]
            # sim-only: the odd col-half's first matmul has start=False; on HW
            # has_written=0 makes it an overwrite, but CoreSim poisons fresh
            # PSUM with NaN — zero it so the += assertion passes.
            for i in range(2):
                nc.vector.memset(accA[i][64:128, :], 0.0)
            panA = stream_panels(p1a, JA, R1, {0: pAg, 1: pAs, 2: pAc}, "pA")
            for k in range(KT):
                half = slice(0, 64) if k % 2 == 0 else slice(64, 128)
                for i in range(2):
                    nc.tensor.matmul(
                        accA[i][half, :],
                        e_sb[:, k * D : (k + 1) * D],
                        panA[k][:, i * 512 : (i + 1) * 512],
                        start=(k == 0),
                        stop=(k >= KT - 2),
                        skip_group_check=(k % 2 == 1),
                    )

            # drain A: stack even/odd halves (same-base copies), transpose
            # each 128-block so the halves land side-by-side in the free dim,
            # then one SBUF add merges them into the t-shard layout.
            with tc.high_priority():
                stA = []
                for i in range(2):
                    st = work.tile([128, 512], F16, name=f"stA{i}", bufs=1)
                    nc.vector.tensor_copy(st[0:64, :], accA[i][0:64, :])
                    nc.vector.tensor_copy(st[64:128, :], accA[i][64:128, :])
                    stA.append(st)
                tshA_sb = work.tile([128, LTA * D], F16, name="tshA", bufs=1)
                for jl in range(LTA):
                    pt = psumT.tile([128, 128], F16, name="ptr")
                    nc.tensor.transpose(
                        pt[:],
                        stA[jl // 4][:, (jl % 4) * 128 : (jl % 4 + 1) * 128],
                        ident_sb[:],
                    )
                    ptx = work.tile([128, 128], F16, name="ptx")
                    nc.vector.tensor_copy(ptx[:], pt[:])
                    nc.vector.tensor_add(
                        tshA_sb[:, jl * D : (jl + 1) * D],
                        ptx[:, 0:64],
                        ptx[:, 64:128],
                    )
                bounceA_in = dram.tile([128, LTA * D], F16)
                nc.scalar.dma_start(bounceA_in[:], tshA_sb[:])
                bounceA_out = dram.tile(
                    [128 * NCORES, LTA * D], F16, addr_space="Shared"
                )
                nc.gpsimd.collective_compute(
                    "AllGather",
                    mybir.AluOpType.bypass,
                    replica_groups=[list(range(NCORES))],
                    ins=[bounceA_in.opt()],
                    outs=[bounceA_out.opt()],
                )

            # ---- phase 1B (256 cols), col-tiled even/odd ----
            accB = psumacc.tile([128, 512], F32, name="accB")
            nc.vector.memset(accB[64:128, :JB], 0.0)
            panB = stream_panels(p1b, JB, R1, {0: pBg, 1: pBs, 2: pBc}, "pB")
            for k in range(KT):
                half = slice(0, 64) if k % 2 == 0 else slice(64, 128)
                nc.tensor.matmul(
                    accB[half, :JB],
                    e_sb[:, k * D : (k + 1) * D],
                    panB[k],
                    start=(k == 0),
                    stop=(k >= KT - 2),
                    skip_group_check=(k % 2 == 1),
                )

            with tc.high_priority():
                stB = work.tile([128, JB], F16, name="stB", bufs=1)
                nc.vector.tensor_copy(stB[0:64, :], accB[0:64, :JB])
                nc.vector.tensor_copy(stB[64:128, :], accB[64:128, :JB])
                tshB_sb = work.tile([128, LTB * D], F16, name="tshB", bufs=1)
                for jl in range(LTB):
                    pt = psumT.tile([128, 128], F16, name="ptr")
                    nc.tensor.transpose(
                        pt[:], stB[:, jl * 128 : (jl + 1) * 128], ident_sb[:]
                    )
                    ptx = work.tile([128, 128], F16, name="ptx")
                    nc.vector.tensor_copy(ptx[:], pt[:])
                    nc.vector.tensor_add(
                        tshB_sb[:, jl * D : (jl + 1) * D],
                        ptx[:, 0:64],
                        ptx[:, 64:128],
                    )
                bounceB_in = dram.tile([128, LTB * D], F16)
                nc.scalar.dma_start(bounceB_in[:], tshB_sb[:])
                bounceB_out = dram.tile(
                    [128 * NCORES, LTB * D], F16, addr_space="Shared"
                )
                nc.gpsimd.collective_compute(
                    "AllGather",
                    mybir.AluOpType.bypass,
                    replica_groups=[list(range(NCORES))],
                    ins=[bounceB_in.opt()],
                    outs=[bounceB_out.opt()],
                )

            # t-table loads ride the SCALAR ring tail (its panels are done):
            # they wait on the AG completions without blocking any stream.
            tA_sb = const.tile([128, NCORES * LTA * D], F16)
            nc.scalar.dma_start(
                tA_sb[:].rearrange("p (r f) -> p r f", r=NCORES),
                bounceA_out[:].rearrange("(r p) f -> p r f", p=128),
            )
            tB_sb = const.tile([128, NCORES * LTB * D], F16)
            nc.scalar.dma_start(
                tB_sb[:].rearrange("p (r f) -> p r f", r=NCORES),
                bounceB_out[:].rearrange("(r p) f -> p r f", p=128),
            )

            # ---- phase 2: h_shard.T = t.T @ p2, col-tiled even/odd pairs ----
            ACCS = ((0, 512), (512, 512), (1024, 256))
            acc_h = [
                psumacc.tile([128, 512], F32, name=f"acch{i}") for i in range(3)
            ]
            for i, (off, w) in enumerate(ACCS):
                nc.vector.memset(acc_h[i][64:128, :w], 0.0)
            pan2 = stream_panels(p2, S, R2, {0: p2g, 1: p2s}, "p2")
            NA = len(TILES_A)  # 64
            for m in range(KT):
                if m < NA:
                    lhsT = tA_sb[:, m * D : (m + 1) * D]
                else:
                    lhsT = tB_sb[:, (m - NA) * D : (m - NA + 1) * D]
                half = slice(0, 64) if m % 2 == 0 else slice(64, 128)
                for i, (off, w) in enumerate(ACCS):
                    nc.tensor.matmul(
                        acc_h[i][half, :w],
                        lhsT,
                        pan2[m][:, off : off + w],
                        start=(m == 0),
                        stop=(m >= KT - 2),
                        skip_group_check=(m % 2 == 1),
                    )

            # ---- drain h (stack halves, transpose, merge) + LayerNorm ----
            stH = []
            for i, (off, w) in enumerate(ACCS):
                st = work.tile([128, w], F16, name=f"stH{i}", bufs=1)
                nc.vector.tensor_copy(st[0:64, :], acc_h[i][0:64, :w])
                nc.vector.tensor_copy(st[64:128, :], acc_h[i][64:128, :w])
                stH.append(st)
            h_sb = work.tile([128, LT * D], F32, name="hsb", bufs=1)
            for r in range(LT):
                i = min(r // 4, 2)
                loc = r * 128 - (0, 512, 1024)[i]
                hp = psumT.tile([128, 128], F16, name="ptr")
                nc.tensor.transpose(
                    hp[:], stH[i][:, loc : loc + 128], ident_sb[:]
                )
                ptx = work.tile([128, 128], F16, name="ptx")
                nc.vector.tensor_copy(ptx[:], hp[:])
                nc.vector.tensor_add(
                    h_sb[:, r * D : (r + 1) * D],
                    ptx[:, 0:64],
                    ptx[:, 64:128],
                )

            h3 = h_sb[:].rearrange("p (r d) -> p r d", d=D)
            nmu = work.tile([128, LT], F32, name="nmu", bufs=1)
            nc.vector.reduce_sum(
                nmu[:], h3, axis=mybir.AxisListType.X, negate=True
            )
            nc.vector.tensor_scalar_mul(nmu[:], nmu[:], 1.0 / D)  # -mu
            hc_sb = work.tile([128, LT * D], F32, name="hc", bufs=1)
            hc3 = hc_sb[:].rearrange("p (r d) -> p r d", d=D)
            nmu3 = nmu[:].rearrange("p (r d) -> p r d", d=1)
            a_b, b_b = bass.broadcast_tensor_aps(h3, nmu3)
            nc.vector.tensor_tensor(hc3, a_b, b_b, op=mybir.AluOpType.add)
            sq_sb = work.tile([128, LT * D], F32, name="sq", bufs=1)
            nc.vector.tensor_mul(sq_sb[:], hc_sb[:], hc_sb[:])
            ssq = work.tile([128, LT], F32, name="ssq", bufs=1)
            nc.vector.reduce_sum(
                ssq[:],
                sq_sb[:].rearrange("p (r d) -> p r d", d=D),
                axis=mybir.AxisListType.X,
            )
            std = work.tile([128, LT], F32, name="std", bufs=1)
            nc.scalar.activation(
                std[:],
                ssq[:],
                mybir.ActivationFunctionType.Sqrt,
                bias=eps_sb[:],
                scale=1.0 / D,
            )
            rstd = work.tile([128, LT], F32, name="rstd", bufs=1)
            nc.vector.reciprocal(rstd[:], std[:])
            o_sb = work.tile([128, LT * D], F32, name="osb", bufs=1)
            o3 = o_sb[:].rearrange("p (r d) -> p r d", d=D)
            rstd3 = rstd[:].rearrange("p (r d) -> p r d", d=1)
            a_b, b_b = bass.broadcast_tensor_aps(hc3, rstd3)
            nc.vector.tensor_tensor(o3, a_b, b_b, op=mybir.AluOpType.mult)
            g3 = gamma_sb[:].rearrange("p (r d) -> p r d", r=1)
            a_b, b_b = bass.broadcast_tensor_aps(o3, g3)
            nc.vector.tensor_tensor(o3, a_b, b_b, op=mybir.AluOpType.mult)
            nc.vector.tensor_add(o_sb[:], o_sb[:], res_sb[:])
            nc.gpsimd.dma_start(
                out.ap().rearrange("(r p) d -> p r d", p=128), o3
            )

    nc.compile()
    _CACHE["nc"] = nc
    return nc


def kernel(
    ego_embeddings,
    adj,
    W_u,
    diag_u,
    par_u,
    W_i,
    diag_i,
    par_i,
    ln_gamma,
    ln_beta,
    trace=False,
):
    global LAST_RUN
    ego = np.ascontiguousarray(ego_embeddings, dtype=np.float32)
    adj = np.ascontiguousarray(adj, dtype=np.float32)
    W_u = np.asarray(W_u, np.float32)
    W_i = np.asarray(W_i, np.float32)
    diag_u = np.asarray(diag_u, np.float32)
    diag_i = np.asarray(diag_i, np.float32)
    gamma = np.asarray(ln_gamma, np.float32)
    beta = np.asarray(ln_beta, np.float32)

    # host phase-0: e = c * (diag*ego) @ W + ego  (42 MFLOP, trivial)
    e = np.empty((N, D), np.float32)
    cu = float(par_u[0]) * float(par_u[1])
    ci = float(par_i[0]) * float(par_i[1])
    e[:NU] = cu * ((diag_u[:, None] * ego[:NU]) @ W_u) + ego[:NU]
    e[NU:] = ci * ((diag_i[:, None] * ego[NU:]) @ W_i) + ego[NU:]
    e16 = e.astype(np.float16)
    e_pre = np.ascontiguousarray(
        e16.reshape(KT, 128, D).transpose(1, 0, 2)
    ).reshape(128, KT * D)

    # LayerNorm(h) is invariant to a global scale on h = adj @ (adj.T @ e):
    # ship adj normalized by its max so the {0, a} graph is EXACTLY {0, 1}
    # in fp8e4, and rescale eps to keep LN bit-faithful.
    scale = float(adj.max())
    if scale <= 0.0:
        scale = 1.0
    inv = np.float32(1.0 / scale)
    eps_dev = np.float32(LN_EPS / (scale * scale * scale * scale))
    adj8 = (adj * inv).astype(ml_dtypes.float8_e4m3)

    gamma_b = np.ascontiguousarray(np.broadcast_to(gamma, (128, D)))
    eps_b = np.full((128, 1), eps_dev, np.float32)

    perm = TILES_A + TILES_B
    in_maps = []
    for c in range(NCORES):
        rows = slice(c * S, (c + 1) * S)
        p2 = np.ascontiguousarray(adj8[rows, :].T)
        p2r = np.ascontiguousarray(
            p2.reshape(KT, 128, S)[perm].reshape(N, S)
        )
        res = ego[rows] + beta[None, :]
        res_pb = np.ascontiguousarray(
            res.reshape(LT, 128, D).transpose(1, 0, 2)
        ).reshape(128, LT * D)
        in_maps.append(
            {
                "p1a": np.ascontiguousarray(adj8[:, c * S : c * S + JA]),
                "p1b": np.ascontiguousarray(adj8[:, c * S + JA : (c + 1) * S]),
                "p2": p2r,
                "e_pre": e_pre,
                "res_pb": res_pb,
                "gamma_b": gamma_b,
                "eps_in": eps_b,
            }
        )

    nc = _build()
    res = bass_utils.run_bass_kernel_spmd(
        nc, in_maps, core_ids=list(range(NCORES)), trace=trace
    )
    LAST_RUN = res
    return np.concatenate([res.results[c]["out"] for c in range(NCORES)], axis=0)


# revision 45
# speedup vs baseline: 1.0993x; 1.0044x over previous
"""HGNN model kernel for Trainium2, 8-core SPMD.

Math (reference):
  e   = par0*par1 * (diag[:,None] * ego) @ W + ego          (per user/item block)
  t   = adj.T @ e
  h   = adj @ t
  out = LayerNorm(h) * gamma + beta + ego

Key structure:
  * adj is {0, a} and LayerNorm is scale-invariant -> panels ship as fp8e4
    {0,1} EXACTLY (half the f16 bytes); LN eps rescaled host-side.
  * e precomputed on host (f16, 1.3 MB); no on-device phase 0. It loads
    first on the sync HWDGE ring (lowest first-byte latency) so the first
    matmul starts within a few microseconds.
  * Panels ride THREE DMA rings (gpsimd/SWDGE ~200 GB/s + sync/scalar
    HWDGE ~110 each) in k-interleaved weighted rounds so panel arrival
    tracks the k-consumption order.
  * Phase-1 j-split 1024/256: AG#1 (1.05 MB) fires at ~1/3 of the run and
    overlaps phase-1B + phase-2A's 64-tile matmul stream; AG#2 (262 KB)
    fires right after phase 1 and overlaps phase-2A. t-table loads ride
    the scalar ring's tail so no panel stream blocks on a collective.
  * PE column-tiling: even/odd k-tiles accumulate into partition halves
    0-63/64-127 of the same PSUM bank via separate moving-operand streams
    (~2x matmul throughput); halves are stacked into SBUF, transposed per
    128-block, and merged by one free-dim add (which also produces the
    transpose needed for the t-shard / LayerNorm layouts).

Sharding: core c owns node rows S*c..S*(c+1) (S = 1280).
"""

import numpy as np
import ml_dtypes

import concourse.bass as bass
import concourse.bacc as bacc
import concourse.tile as tile
from concourse import bass_utils, mybir
from concourse.masks import make_identity

F32 = mybir.dt.float32
F16 = mybir.dt.float16
F8 = mybir.dt.float8e4

N = 10240
D = 64
NU = 4096
NCORES = 8
S = N // NCORES          # 1280 rows per core
KT = N // 128            # 80 global 128-row k-tiles
LT = S // 128            # 10 local 128-row tiles
LN_EPS = 1e-5

JA = 1024                # phase-1 j-split: half A columns (8 local tiles)
JB = S - JA              # 256 (2 local tiles)
LTA = JA // 128          # 8
LTB = JB // 128          # 2

# phase-2 consumes t tiles in AllGather arrival order.
TILES_A = [10 * c + q for c in range(NCORES) for q in range(LTA)]
TILES_B = [10 * c + q for c in range(NCORES) for q in range(LTA, LT)]

# k-interleaved DMA rounds: (engine_idx, k-tile count) per round slice.
# engine 0 = gpsimd (fast SWDGE), 1 = sync, 2 = scalar.
# scalar leads each round: an empty HWDGE ring starts in ~2 us, so the
# k=0 panel lands long before the gpsimd/SWDGE ring warms up.
R1 = [(2, 4), (0, 8), (1, 4)]          # phase-1 rounds: 16 k-tiles
R2 = [(0, 10), (1, 6)]                 # phase-2 rounds: 16 k-tiles (no scalar)

_CACHE = {}
LAST_RUN = None  # BassKernelResults of the most recent execution (for test.py)


def _build():
    if "nc" in _CACHE:
        return _CACHE["nc"]

    nc = bacc.Bacc(
        "TRN2",
        target_bir_lowering=False,
        debug=False,
        enable_asserts=True,
        num_devices=NCORES,
    )

    p1a = nc.dram_tensor("p1a", [N, JA], F8, kind="ExternalInput")
    p1b = nc.dram_tensor("p1b", [N, JB], F8, kind="ExternalInput")
    p2 = nc.dram_tensor("p2", [N, S], F8, kind="ExternalInput")
    e_pre = nc.dram_tensor("e_pre", [128, KT * D], F16, kind="ExternalInput")
    res_pb = nc.dram_tensor("res_pb", [128, LT * D], F32, kind="ExternalInput")
    gamma_b = nc.dram_tensor("gamma_b", [128, D], F32, kind="ExternalInput")
    eps_in = nc.dram_tensor("eps_in", [128, 1], F32, kind="ExternalInput")
    out = nc.dram_tensor("out", [S, D], F32, kind="ExternalOutput")

    with tile.TileContext(nc) as tc:
        with (
            tc.tile_pool(name="const", bufs=1) as const,
            tc.tile_pool(name="pAg", bufs=2) as pAg,
            tc.tile_pool(name="pAs", bufs=3) as pAs,
            tc.tile_pool(name="pAc", bufs=3) as pAc,
            # B panels fully buffered (tiny): their transfers finish right
            # after the A stream, so the scalar ring reaches the AllGather
            # bounce ~15 us earlier instead of draining PE-gated batches.
            tc.tile_pool(name="pBg", bufs=5) as pBg,
            tc.tile_pool(name="pBs", bufs=5) as pBs,
            tc.tile_pool(name="pBc", bufs=5) as pBc,
            tc.tile_pool(name="p2g", bufs=2) as p2g,
            tc.tile_pool(name="p2s", bufs=2) as p2s,
            tc.tile_pool(name="work", bufs=2) as work,
            tc.tile_pool(name="psumT", bufs=2, space="PSUM") as psumT,
            tc.tile_pool(name="psumacc", bufs=1, space="PSUM") as psumacc,
            tc.tile_pool(name="dram", bufs=1, space="DRAM") as dram,
        ):
            ENGS = [nc.gpsimd, nc.sync, nc.scalar]

            # ---- constants; e FIRST on the empty sync HWDGE ring ----
            e_sb = const.tile([128, KT * D], F16)
            with tc.high_priority():
                nc.sync.dma_start(e_sb[:], e_pre.ap())
            res_sb = const.tile([128, LT * D], F32)
            nc.gpsimd.dma_start(res_sb[:], res_pb.ap())
            gamma_sb = const.tile([128, D], F32)
            nc.gpsimd.dma_start(gamma_sb[:], gamma_b.ap())
            eps_sb = const.tile([128, 1], F32)
            nc.gpsimd.dma_start(eps_sb[:], eps_in.ap())
            ident_sb = const.tile([128, 128], F16)
            make_identity(nc, ident_sb[:])

            def stream_panels(dram_t, width, rounds, pools, name):
                """Issue k-interleaved panel DMAs; return k -> panel AP."""
                kv = dram_t.ap().rearrange("(k p) j -> k p j", p=128)
                total = sum(n for _, n in rounds)
                assert KT % total == 0
                sched = []  # (k0, n, engine_idx)
                base = 0
                while base < KT:
                    off = 0
                    for ei, cnt in rounds:
                        sched.append((base + off, cnt, ei))
                        off += cnt
                    base += total
                tiles = {}
                for k0, cnt, ei in sched:
                    pan = pools[ei].tile(
                        [128, cnt * width], F8, name=f"{name}{ei}"
                    )
                    ENGS[ei].dma_start(
                        pan[:].rearrange("p (t j) -> p t j", j=width),
                        kv[k0 : k0 + cnt].rearrange("k p j -> p k j"),
                    )
                    for t_i in range(cnt):
                        tiles[k0 + t_i] = pan[
                            :, t_i * width : (t_i + 1) * width
                        ]
                return tiles

            # ---- phase 1A: t[:,0:1024].T = e.T @ p1a, col-tiled even/odd ----
            accA = [psumacc.tile([128, 512], F32, name=f"accA{i}") for i in range(2)]
            # sim-only: the odd col-half's first matmul has start=False; on HW
            # has_written=0 makes it an overwrite, but CoreSim poisons fresh
            # PSUM with NaN — zero it so the += assertion passes.
            for i in range(2):
                nc.vector.memset(accA[i][64:128, :], 0.0)
            panA = stream_panels(p1a, JA, R1, {0: pAg, 1: pAs, 2: pAc}, "pA")
            for k in range(KT):
                half = slice(0, 64) if k % 2 == 0 else slice(64, 128)
                for i in range(2):
                    nc.tensor.matmul(
                        accA[i][half, :],
                        e_sb[:, k * D : (k + 1) * D],
                        panA[k][:, i * 512 : (i + 1) * 512],
                        start=(k == 0),
                        stop=(k >= KT - 2),
                        skip_group_check=(k % 2 == 1),
                    )

            # drain A: stack even/odd halves (same-base copies), transpose
            # each 128-block so the halves land side-by-side in the free dim,
            # then one SBUF add merges them into the t-shard layout.
            with tc.high_priority():
                stA = []
                for i in range(2):
                    st = work.tile([128, 512], F16, name=f"stA{i}", bufs=1)
                    nc.vector.tensor_copy(st[0:64, :], accA[i][0:64, :])
                    nc.vector.tensor_copy(st[64:128, :], accA[i][64:128, :])
                    stA.append(st)
                tshA_sb = work.tile([128, LTA * D], F16, name="tshA", bufs=1)
                for jl in range(LTA):
                    pt = psumT.tile([128, 128], F16, name="ptr")
                    nc.tensor.transpose(
                        pt[:],
                        stA[jl // 4][:, (jl % 4) * 128 : (jl % 4 + 1) * 128],
                        ident_sb[:],
                    )
                    ptx = work.tile([128, 128], F16, name="ptx")
                    nc.vector.tensor_copy(ptx[:], pt[:])
                    nc.vector.tensor_add(
                        tshA_sb[:, jl * D : (jl + 1) * D],
                        ptx[:, 0:64],
                        ptx[:, 64:128],
                    )
                bounceA_in = dram.tile([128, LTA * D], F16)
                nc.scalar.dma_start(bounceA_in[:], tshA_sb[:])
                bounceA_out = dram.tile(
                    [128 * NCORES, LTA * D], F16, addr_space="Shared"
                )
                nc.gpsimd.collective_compute(
                    "AllGather",
                    mybir.AluOpType.bypass,
                    replica_groups=[list(range(NCORES))],
                    ins=[bounceA_in.opt()],
                    outs=[bounceA_out.opt()],
                )

            # ---- phase 1B (256 cols), col-tiled even/odd ----
            accB = psumacc.tile([128, 512], F32, name="accB")
            nc.vector.memset(accB[64:128, :JB], 0.0)
            panB = stream_panels(p1b, JB, R1, {0: pBg, 1: pBs, 2: pBc}, "pB")
            for k in range(KT):
                half = slice(0, 64) if k % 2 == 0 else slice(64, 128)
                nc.tensor.matmul(
                    accB[half, :JB],
                    e_sb[:, k * D : (k + 1) * D],
                    panB[k],
                    start=(k == 0),
                    stop=(k >= KT - 2),
                    skip_group_check=(k % 2 == 1),
                )

            with tc.high_priority():
                stB = work.tile([128, JB], F16, name="stB", bufs=1)
                nc.vector.tensor_copy(stB[0:64, :], accB[0:64, :JB])
                nc.vector.tensor_copy(stB[64:128, :], accB[64:128, :JB])
                tshB_sb = work.tile([128, LTB * D], F16, name="tshB", bufs=1)
                for jl in range(LTB):
                    pt = psumT.tile([128, 128], F16, name="ptr")
                    nc.tensor.transpose(
                        pt[:], stB[:, jl * 128 : (jl + 1) * 128], ident_sb[:]
                    )
                    ptx = work.tile([128, 128], F16, name="ptx")
                    nc.vector.tensor_copy(ptx[:], pt[:])
                    nc.vector.tensor_add(
                        tshB_sb[:, jl * D : (jl + 1) * D],
                        ptx[:, 0:64],
                        ptx[:, 64:128],
                    )
                bounceB_in = dram.tile([128, LTB * D], F16)
                nc.scalar.dma_start(bounceB_in[:], tshB_sb[:])
                bounceB_out = dram.tile(
                    [128 * NCORES, LTB * D], F16, addr_space="Shared"
                )
                nc.gpsimd.collective_compute(
                    "AllGather",
                    mybir.AluOpType.bypass,
                    replica_groups=[list(range(NCORES))],
                    ins=[bounceB_in.opt()],
                    outs=[bounceB_out.opt()],
                )

            # t-table loads ride the SCALAR ring tail (its panels are done):
            # they wait on the AG completions without blocking any stream.
            tA_sb = const.tile([128, NCORES * LTA * D], F16)
            nc.scalar.dma_start(
                tA_sb[:].rearrange("p (r f) -> p r f", r=NCORES),
                bounceA_out[:].rearrange("(r p) f -> p r f", p=128),
            )
            tB_sb = const.tile([128, NCORES * LTB * D], F16)
            nc.scalar.dma_start(
                tB_sb[:].rearrange("p (r f) -> p r f", r=NCORES),
                bounceB_out[:].rearrange("(r p) f -> p r f", p=128),
            )

            # ---- phase 2: h_shard.T = t.T @ p2, col-tiled even/odd pairs ----
            ACCS = ((0, 512), (512, 512), (1024, 256))
            acc_h = [
                psumacc.tile([128, 512], F32, name=f"acch{i}") for i in range(3)
            ]
            for i, (off, w) in enumerate(ACCS):
                nc.vector.memset(acc_h[i][64:128, :w], 0.0)
            pan2 = stream_panels(p2, S, R2, {0: p2g, 1: p2s}, "p2")
            NA = len(TILES_A)  # 64
            for m in range(KT):
                if m < NA:
                    lhsT = tA_sb[:, m * D : (m + 1) * D]
                else:
                    lhsT = tB_sb[:, (m - NA) * D : (m - NA + 1) * D]
                half = slice(0, 64) if m % 2 == 0 else slice(64, 128)
                for i, (off, w) in enumerate(ACCS):
                    nc.tensor.matmul(
                        acc_h[i][half, :w],
                        lhsT,
                        pan2[m][:, off : off + w],
                        start=(m == 0),
                        stop=(m >= KT - 2),
                        skip_group_check=(m % 2 == 1),
                    )

            # ---- drain h (stack halves, transpose, merge) + LayerNorm ----
            stH = []
            for i, (off, w) in enumerate(ACCS):
                st = work.tile([128, w], F16, name=f"stH{i}", bufs=1)
                nc.vector.tensor_copy(st[0:64, :], acc_h[i][0:64, :w])
                nc.vector.tensor_copy(st[64:128, :], acc_h[i][64:128, :w])
                stH.append(st)
            h_sb = work.tile([128, LT * D], F32, name="hsb", bufs=1)
            for r in range(LT):
                i = min(r // 4, 2)
                loc = r * 128 - (0, 512, 1024)[i]
                hp = psumT.tile([128, 128], F16, name="ptr")
                nc.tensor.transpose(
                    hp[:], stH[i][:, loc : loc + 128], ident_sb[:]
                )
                ptx = work.tile([128, 128], F16, name="ptx")
                nc.vector.tensor_copy(ptx[:], hp[:])
                nc.vector.tensor_add(
                    h_sb[:, r * D : (r + 1) * D],
                    ptx[:, 0:64],
                    ptx[:, 64:128],
                )

            h3 = h_sb[:].rearrange("p (r d) -> p r d", d=D)
            nmu = work.tile([128, LT], F32, name="nmu", bufs=1)
            nc.vector.reduce_sum(
                nmu[:], h3, axis=mybir.AxisListType.X, negate=True
            )
            nc.vector.tensor_scalar_mul(nmu[:], nmu[:], 1.0 / D)  # -mu
            hc_sb = work.tile([128, LT * D], F32, name="hc", bufs=1)
            hc3 = hc_sb[:].rearrange("p (r d) -> p r d", d=D)
            nmu3 = nmu[:].rearrange("p (r d) -> p r d", d=1)
            a_b, b_b = bass.broadcast_tensor_aps(h3, nmu3)
            nc.vector.tensor_tensor(hc3, a_b, b_b, op=mybir.AluOpType.add)
            sq_sb = work.tile([128, LT * D], F32, name="sq", bufs=1)
            nc.vector.tensor_mul(sq_sb[:], hc_sb[:], hc_sb[:])
            ssq = work.tile([128, LT], F32, name="ssq", bufs=1)
            nc.vector.reduce_sum(
                ssq[:],
                sq_sb[:].rearrange("p (r d) -> p r d", d=D),
                axis=mybir.AxisListType.X,
            )
            std = work.tile([128, LT], F32, name="std", bufs=1)
            nc.scalar.activation(
                std[:],
                ssq[:],
                mybir.ActivationFunctionType.Sqrt,
                bias=eps_sb[:],
                scale=1.0 / D,
            )
            rstd = work.tile([128, LT], F32, name="rstd", bufs=1)
            nc.vector.reciprocal(rstd[:], std[:])
            o_sb = work.tile([128, LT * D], F32, name="osb", bufs=1)
            o3 = o_sb[:].rearrange("p (r d) -> p r d", d=D)
            rstd3 = rstd[:].rearrange("p (r d) -> p r d", d=1)
            a_b, b_b = bass.broadcast_tensor_aps(hc3, rstd3)
            nc.vector.tensor_tensor(o3, a_b, b_b, op=mybir.AluOpType.mult)
            g3 = gamma_sb[:].rearrange("p (r d) -> p r d", r=1)
            a_b, b_b = bass.broadcast_tensor_aps(o3, g3)
            nc.vector.tensor_tensor(o3, a_b, b_b, op=mybir.AluOpType.mult)
            nc.vector.tensor_add(o_sb[:], o_sb[:], res_sb[:])
            nc.gpsimd.dma_start(
                out.ap().rearrange("(r p) d -> p r d", p=128), o3
            )

    nc.compile()
    _CACHE["nc"] = nc
    return nc


def kernel(
    ego_embeddings,
    adj,
    W_u,
    diag_u,
    par_u,
    W_i,
    diag_i,
    par_i,
    ln_gamma,
    ln_beta,
    trace=False,
):
    global LAST_RUN
    ego = np.ascontiguousarray(ego_embeddings, dtype=np.float32)
    adj = np.ascontiguousarray(adj, dtype=np.float32)
    W_u = np.asarray(W_u, np.float32)
    W_i = np.asarray(W_i, np.float32)
    diag_u = np.asarray(diag_u, np.float32)
    diag_i = np.asarray(diag_i, np.float32)
    gamma = np.asarray(ln_gamma, np.float32)
    beta = np.asarray(ln_beta, np.float32)

    # host phase-0: e = c * (diag*ego) @ W + ego  (42 MFLOP, trivial)
    e = np.empty((N, D), np.float32)
    cu = float(par_u[0]) * float(par_u[1])
    ci = float(par_i[0]) * float(par_i[1])
    e[:NU] = cu * ((diag_u[:, None] * ego[:NU]) @ W_u) + ego[:NU]
    e[NU:] = ci * ((diag_i[:, None] * ego[NU:]) @ W_i) + ego[NU:]
    e16 = e.astype(np.float16)
    e_pre = np.ascontiguousarray(
        e16.reshape(KT, 128, D).transpose(1, 0, 2)
    ).reshape(128, KT * D)

    # LayerNorm(h) is invariant to a global scale on h = adj @ (adj.T @ e):
    # ship adj normalized by its max so the {0, a} graph is EXACTLY {0, 1}
    # in fp8e4, and rescale eps to keep LN bit-faithful.
    scale = float(adj.max())
    if scale <= 0.0:
        scale = 1.0
    inv = np.float32(1.0 / scale)
    eps_dev = np.float32(LN_EPS / (scale * scale * scale * scale))
    adj8 = (adj * inv).astype(ml_dtypes.float8_e4m3)

    gamma_b = np.ascontiguousarray(np.broadcast_to(gamma, (128, D)))
    eps_b = np.full((128, 1), eps_dev, np.float32)

    perm = TILES_A + TILES_B
    in_maps = []
    for c in range(NCORES):
        rows = slice(c * S, (c + 1) * S)
        p2 = np.ascontiguousarray(adj8[rows, :].T)
        p2r = np.ascontiguousarray(
            p2.reshape(KT, 128, S)[perm].reshape(N, S)
        )
        res = ego[rows] + beta[None, :]
        res_pb = np.ascontiguousarray(
            res.reshape(LT, 128, D).transpose(1, 0, 2)
        ).reshape(128, LT * D)
        in_maps.append(
            {
                "p1a": np.ascontiguousarray(adj8[:, c * S : c * S + JA]),
                "p1b": np.ascontiguousarray(adj8[:, c * S + JA : (c + 1) * S]),
                "p2": p2r,
                "e_pre": e_pre,
                "res_pb": res_pb,
                "gamma_b": gamma_b,
                "eps_in": eps_b,
            }
        )

    nc = _build()
    res = bass_utils.run_bass_kernel_spmd(
        nc, in_maps, core_ids=list(range(NCORES)), trace=trace
    )
    LAST_RUN = res
    return np.concatenate([res.results[c]["out"] for c in range(NCORES)], axis=0)
